# revision 2
# baseline (speedup 1.0000x reference)
"""GPNConv (GNN message passing) Trainium2 Bass kernel.

  agg = segment_sum(x[col], row, N)        # [N, 128]
  out = (x + agg) @ W.T + b                # [N, 512]

Sharding: destination nodes split across 8 cores (12500 each); no
cross-core communication. Per core, edges are grouped by 256-node
destination window ("pair") and by 32768-row source bucket (dma_gather
indices are int16). Each core bulk-gathers its neighbor rows x[col]
from a replicated x via dma_gather, segment-sums them with one-hot
matmuls on the PE (one-hot built on-chip from dest slots via is_equal
against a host-supplied iota row), adds the residual from a
host-transposed x-shard, applies the 128->512 linear + bias, and
writes its output shard.

The chunk schedule (how many 128-edge chunks each (pair, bucket) group
gets) is data-dependent but made uniform across cores by taking the max
over cores, so one SPMD program serves all 8 cores.
"""

import numpy as np

import concourse.bass as bass
import concourse.mybir as mybir
import concourse.tile as tile
from concourse import bacc
from concourse import bass_utils

P = 128
N_NODES = 100000
D_IN = 128
D_OUT = 512
N_CORES = 8
NODES_PER_CORE = N_NODES // N_CORES             # 12500
DPAIR = 256                                      # dest window (psum free dim)
PAIRS_PER_CORE = (NODES_PER_CORE + DPAIR - 1) // DPAIR  # 49
PAD_NODES = PAIRS_PER_CORE * DPAIR               # 12544
WAVE_PAIRS = 7                                   # pairs per gather wave
N_WAVES = (PAIRS_PER_CORE + WAVE_PAIRS - 1) // WAVE_PAIRS  # 7
BUCKET = 32768                                   # int16 index range
PAD_SLOT = 999.0                                 # one-hot slot matching nothing

_F32 = mybir.dt.float32
_BF16 = mybir.dt.bfloat16
_I16 = mybir.dt.int16


def _host_prep(edge_index):
    """Group edges by (core, pair, bucket); build uniform chunk schedule,
    int16 gather-index array and f32 dest-slot array per core."""
    row = np.asarray(edge_index[0], dtype=np.int64)
    col = np.asarray(edge_index[1], dtype=np.int64)
    n_buckets = (N_NODES + BUCKET - 1) // BUCKET  # 4

    core = row // NODES_PER_CORE
    local = row % NODES_PER_CORE
    pair = local // DPAIR                         # 0..48
    pslot = local % DPAIR                         # 0..255
    bucket = col // BUCKET
    brel = (col % BUCKET).astype(np.int16)

    # group key and counts
    key = (core * PAIRS_PER_CORE + pair) * n_buckets + bucket
    ngroups = N_CORES * PAIRS_PER_CORE * n_buckets
    counts = np.bincount(key, minlength=ngroups).reshape(
        N_CORES, PAIRS_PER_CORE, n_buckets
    )
    budget = -(-counts.max(axis=0) // P)          # [PAIRS, NB] ceil
    budget[:, 0] = np.maximum(budget[:, 0], 1)    # >=1 chunk per pair

    order = np.argsort(key, kind="stable")
    brel_s = brel[order]
    pslot_s = pslot[order]
    key_s = key[order]
    starts = np.searchsorted(key_s, np.arange(ngroups + 1))

    # chunk stream: for wave w, for bucket b, for pair p in wave, chunks
    waves = [
        list(range(w * WAVE_PAIRS, min((w + 1) * WAVE_PAIRS, PAIRS_PER_CORE)))
        for w in range(N_WAVES)
    ]
    # schedule structures (identical across cores)
    gathers = []      # per (w,b): dict(nch, qoff, coff, bucket, wave)
    pair_chunks = {}  # pair -> list of (w, b, local_chunk_in_gather, ci)
    ci = 0            # global chunk counter
    qcols = 0         # idx tile columns consumed (num_idxs/16 each)
    for w, wp in enumerate(waves):
        for b in range(n_buckets):
            nch = int(sum(budget[p][b] for p in wp))
            if nch == 0:
                continue
            g = dict(w=w, b=b, nch=nch, qoff=qcols, coff=ci)
            gathers.append(g)
            lc = 0
            for p in wp:
                for j in range(int(budget[p][b])):
                    pair_chunks.setdefault(p, []).append((len(gathers) - 1, lc, ci))
                    lc += 1
                    ci += 1
            qcols += nch * 8  # (nch*128 idxs)/16
    TC = ci

    idx_all = np.zeros((N_CORES, 16, qcols), dtype=np.int16)
    dfl_all = np.full((N_CORES, P, TC), PAD_SLOT, dtype=np.float32)
    for c in range(N_CORES):
        for g in gathers:
            w, b = g["w"], g["b"]
            lc = 0
            for p in waves[w]:
                gk = (c * PAIRS_PER_CORE + p) * n_buckets + b
                b0, b1 = starts[gk], starts[gk + 1]
                n = b1 - b0
                kb = int(budget[p][b])
                assert n <= kb * P
                if n:
                    # edge j -> gather idx position i = (lc + j//128)*128 + j%128
                    i = (lc + np.arange(n) // P) * P + np.arange(n) % P
                    idx_all[c, i % 16, g["qoff"] + i // 16] = brel_s[b0:b1]
                    dfl_all[c, np.arange(n) % P, g["coff"] + lc + np.arange(n) // P] = (
                        pslot_s[b0:b1]
                    )
                lc += kb
    # replicate idx rows to 128 partitions (8 Q7 cores x 16-partition stripes)
    idx_all = np.tile(idx_all, (1, 8, 1))
    return idx_all, dfl_all, gathers, pair_chunks, waves, TC, qcols, n_buckets


def _build_program(gathers, pair_chunks, waves, TC, qcols, n_buckets):
    nc = bacc.Bacc(
        "TRN2",
        target_bir_lowering=False,
        debug=False,
        enable_asserts=False,
        num_devices=N_CORES,
    )
    x_d = nc.dram_tensor("x", [N_NODES, D_IN], _BF16, kind="ExternalInput").ap()
    idx_d = nc.dram_tensor("idx", [P, qcols], _I16, kind="ExternalInput").ap()
    dfl_d = nc.dram_tensor("dfl", [P, TC], _F32, kind="ExternalInput").ap()
    xt_d = nc.dram_tensor("xt", [P, PAD_NODES], _F32, kind="ExternalInput").ap()
    wt_d = nc.dram_tensor("wt", [P, D_OUT], _F32, kind="ExternalInput").ap()
    bias_d = nc.dram_tensor("bias", [P, D_OUT], _F32, kind="ExternalInput").ap()
    iota_d = nc.dram_tensor("iota", [P, DPAIR], _F32, kind="ExternalInput").ap()
    out_d = nc.dram_tensor("out", [PAD_NODES, D_OUT], _F32, kind="ExternalOutput").ap()

    with tile.TileContext(nc) as tc:
        with (
            tc.tile_pool(name="const", bufs=1) as cpool,
            tc.tile_pool(name="gather", bufs=2) as gpool,
            tc.tile_pool(name="xtw", bufs=2) as xtpool,
            tc.tile_pool(name="oh", bufs=6) as ohpool,
            tc.tile_pool(name="ht", bufs=3) as htpool,
            tc.tile_pool(name="ot", bufs=3) as otpool,
            tc.tile_pool(name="psA", bufs=4, space="PSUM") as psA,
            tc.tile_pool(name="psB", bufs=2, space="PSUM") as psB,
        ):
            wt_t = cpool.tile([P, D_OUT], _F32)
            nc.sync.dma_start(out=wt_t[:], in_=wt_d)
            bias_t = cpool.tile([P, D_OUT], _F32)
            nc.sync.dma_start(out=bias_t[:], in_=bias_d)
            iota_t = cpool.tile([P, DPAIR], _F32)
            nc.sync.dma_start(out=iota_t[:], in_=iota_d)
            idx_t = cpool.tile([P, qcols], _I16)
            nc.sync.dma_start(out=idx_t[:], in_=idx_d)
            dfl_t = cpool.tile([P, TC], _F32)
            nc.sync.dma_start(out=dfl_t[:], in_=dfl_d)

            for w, wp in enumerate(waves):
                gts = {}
                for g in gathers:
                    if g["w"] != w:
                        continue
                    b = g["b"]
                    nch = g["nch"]
                    b0 = b * BUCKET
                    b1 = min(b0 + BUCKET, N_NODES)
                    gt = gpool.tile([P, nch * P], _BF16, tag=f"g{b}")
                    nc.gpsimd.dma_gather(
                        gt[:].rearrange("p (c e) -> p c e", e=P),
                        x_d[b0:b1, :],
                        idx_t[:, g["qoff"] : g["qoff"] + nch * 8],
                        nch * P,
                        nch * P,
                        P,
                        single_packet=False,
                    )
                    gts[b] = gt
                xtw = xtpool.tile([P, len(wp) * DPAIR], _F32)
                nc.sync.dma_start(
                    out=xtw[:],
                    in_=xt_d[:, wp[0] * DPAIR : (wp[-1] + 1) * DPAIR],
                )
                for pi, p in enumerate(wp):
                    chunks = pair_chunks[p]
                    psT = psA.tile([P, DPAIR], _F32)
                    for k, (gi, lc, ci) in enumerate(chunks):
                        oh = ohpool.tile([P, DPAIR], _BF16)
                        nc.vector.tensor_scalar(
                            out=oh[:],
                            in0=iota_t[:],
                            scalar1=dfl_t[:, ci : ci + 1],
                            scalar2=None,
                            op0=mybir.AluOpType.is_equal,
                        )
                        nc.tensor.matmul(
                            out=psT[:],
                            lhsT=gts[gathers[gi]["b"]][:, lc * P : (lc + 1) * P],
                            rhs=oh[:],
                            start=(k == 0),
                            stop=(k == len(chunks) - 1),
                        )
                    for h in range(2):
                        ht = htpool.tile([P, P], _F32)
                        nc.vector.tensor_add(
                            out=ht[:],
                            in0=psT[:, h * P : (h + 1) * P],
                            in1=xtw[:, (pi * DPAIR + h * P) : (pi * DPAIR + (h + 1) * P)],
                        )
                        psO = psB.tile([P, D_OUT], _F32)
                        nc.tensor.matmul(
                            out=psO[:], lhsT=ht[:], rhs=wt_t[:], start=True, stop=True
                        )
                        ot = otpool.tile([P, D_OUT], _F32)
                        nc.vector.tensor_add(out=ot[:], in0=psO[:], in1=bias_t[:])
                        r0 = p * DPAIR + h * P
                        nc.sync.dma_start(out=out_d[r0 : r0 + P, :], in_=ot[:])
    nc.compile()
    return nc


def _prepare(inputs):
    """Host prep + program build: returns (nc, in_maps)."""
    import ml_dtypes
    x = np.ascontiguousarray(np.asarray(inputs["x"], dtype=np.float32))
    xb = np.ascontiguousarray(x.astype(ml_dtypes.bfloat16))
    W = np.asarray(inputs["W"], dtype=np.float32)
    b = np.asarray(inputs["b"], dtype=np.float32)

    idx_all, dfl_all, gathers, pair_chunks, waves, TC, qcols, _nb = _host_prep(
        inputs["edge_index"]
    )

    WT = np.ascontiguousarray(W.T)
    bias_rep = np.ascontiguousarray(np.broadcast_to(b[None, :], (P, D_OUT))).astype(
        np.float32
    )
    iota = np.ascontiguousarray(
        np.broadcast_to(np.arange(DPAIR, dtype=np.float32)[None, :], (P, DPAIR))
    )

    in_maps = []
    for c in range(N_CORES):
        xt = np.zeros((P, PAD_NODES), dtype=np.float32)
        xt[:, :NODES_PER_CORE] = x[c * NODES_PER_CORE : (c + 1) * NODES_PER_CORE].T
        in_maps.append(
            {
                "x": xb,
                "idx": np.ascontiguousarray(idx_all[c]),
                "dfl": np.ascontiguousarray(dfl_all[c]),
                "xt": xt,
                "wt": WT,
                "bias": bias_rep,
                "iota": iota,
            }
        )

    nc = _build_program(gathers, pair_chunks, waves, TC, qcols, _nb)
    return nc, in_maps


def _run(inputs, trace=False, prepared=None):
    nc, in_maps = prepared if prepared is not None else _prepare(inputs)
    res = bass_utils.run_bass_kernel_spmd(
        nc, in_maps, core_ids=list(range(N_CORES)), trace=trace
    )
    out = np.concatenate(
        [res.results[c]["out"][:NODES_PER_CORE] for c in range(N_CORES)], axis=0
    )
    return out.astype(np.float32), res


def kernel(**inputs):
    out, _ = _run(inputs, trace=False)
    return out



# revision 27
# speedup vs baseline: 1.0612x; 1.0612x over previous
"""GPNConv (GNN message passing) Trainium2 Bass kernel.

  agg = segment_sum(x[col], row, N)        # [N, 128]
  out = (x + agg) @ W.T + b                # [N, 512]

Sharding: destination nodes split across 8 cores (12500 each); no
cross-core communication. Per core, edges are grouped by 256-node
destination window ("pair") and by 25000-row source bucket (dma_gather
indices are int16). Each core bulk-gathers its neighbor rows x[col]
from a replicated x via dma_gather (4 SWDGE queues, one per bucket),
segment-sums them with one-hot matmuls on the PE (one-hot built on DVE
from dest slots via is_equal against an iota row, all bf16), adds the
residual via PE identity matmuls from a per-core natural x-shard, folds
the bias in as a rank-1 PE matmul, copies PSUM->SBUF on the scalar
engine, applies the 128->512 linear from bf16 tiles, and DMAs each
output block straight from PSUM.

The chunk schedule (how many 128-edge chunks each (pair, bucket) group
gets) is data-dependent but made uniform across cores by taking the max
over cores, so one SPMD program serves all 8 cores.
"""

import numpy as np

import concourse.bass as bass
import concourse.mybir as mybir
import concourse.tile as tile
from concourse import bacc
from concourse import bass_utils

P = 128
N_NODES = 100000
D_IN = 128
D_OUT = 512
N_CORES = 8
NODES_PER_CORE = N_NODES // N_CORES             # 12500
DPAIR = 256                                      # dest window (psum free dim)
PAIRS_PER_CORE = (NODES_PER_CORE + DPAIR - 1) // DPAIR  # 49
PAD_NODES = PAIRS_PER_CORE * DPAIR               # 12544
WAVE_PAIRS = 7                                   # pairs per gather wave
N_WAVES = (PAIRS_PER_CORE + WAVE_PAIRS - 1) // WAVE_PAIRS  # 7
N_BUCKETS = 4
BOUNDS = (0, 28800, 57600, 86400)                # source bucket starts (int16 range)
N_QUEUES = 4                                     # SWDGE queues (one per bucket)
PAD_SLOT = 300.0                                 # one-hot slot matching nothing

_F32 = mybir.dt.float32
_BF16 = mybir.dt.bfloat16
_I16 = mybir.dt.int16


def _host_prep(edge_index):
    """Group edges by (core, pair, bucket); build uniform chunk schedule,
    int16 gather-index array and bf16 dest-slot array per core."""
    row = np.asarray(edge_index[0], dtype=np.int64)
    col = np.asarray(edge_index[1], dtype=np.int64)

    core = row // NODES_PER_CORE
    local = row % NODES_PER_CORE
    pair = local // DPAIR                         # 0..48
    pslot = local % DPAIR                         # 0..255
    bounds = np.asarray(BOUNDS)
    bucket = np.searchsorted(bounds, col, side="right") - 1  # 0..3
    brel = (col - bounds[bucket]).astype(np.int16)

    key = (core * PAIRS_PER_CORE + pair) * N_BUCKETS + bucket
    ngroups = N_CORES * PAIRS_PER_CORE * N_BUCKETS
    counts = np.bincount(key, minlength=ngroups).reshape(
        N_CORES, PAIRS_PER_CORE, N_BUCKETS
    )
    budget = -(-counts.max(axis=0) // P)          # [PAIRS, NB] ceil

    order = np.argsort(key, kind="stable")
    brel_s = brel[order]
    pslot_s = pslot[order]
    key_s = key[order]
    starts = np.searchsorted(key_s, np.arange(ngroups + 1))

    waves = [
        list(range(w * WAVE_PAIRS, min((w + 1) * WAVE_PAIRS, PAIRS_PER_CORE)))
        for w in range(N_WAVES)
    ]
    gathers = []      # per (w,b): dict(nch, qoff, coff, b, w)
    pair_chunks = {p: [] for p in range(PAIRS_PER_CORE)}
    ci = 0            # global chunk counter
    qcols = 0         # idx tile columns consumed (num_idxs/16 each)
    for w, wp in enumerate(waves):
        for b in range(N_BUCKETS):
            nch = int(sum(budget[p][b] for p in wp))
            if nch == 0:
                continue
            gathers.append(dict(w=w, b=b, nch=nch, qoff=qcols, coff=ci))
            lc = 0
            for p in wp:
                for j in range(int(budget[p][b])):
                    pair_chunks[p].append((len(gathers) - 1, lc, ci))
                    lc += 1
                    ci += 1
            qcols += nch * 8  # (nch*128 idxs)/16
    TC = max(ci, 1)

    idx_all = np.zeros((N_CORES, 16, max(qcols, 8)), dtype=np.int16)
    dfl_all = np.full((N_CORES, P, TC), PAD_SLOT, dtype=np.float32)
    for c in range(N_CORES):
        for g in gathers:
            w, b = g["w"], g["b"]
            lc = 0
            for p in waves[w]:
                gk = (c * PAIRS_PER_CORE + p) * N_BUCKETS + b
                b0, b1 = starts[gk], starts[gk + 1]
                n = b1 - b0
                kb = int(budget[p][b])
                assert n <= kb * P
                if n:
                    i = (lc + np.arange(n) // P) * P + np.arange(n) % P
                    idx_all[c, i % 16, g["qoff"] + i // 16] = brel_s[b0:b1]
                    dfl_all[c, np.arange(n) % P, g["coff"] + lc + np.arange(n) // P] = (
                        pslot_s[b0:b1]
                    )
                lc += kb
    # replicate idx rows to 128 partitions (8 Q7 cores x 16-partition stripes)
    idx_all = np.tile(idx_all, (1, 8, 1))
    return idx_all, dfl_all, gathers, pair_chunks, waves, TC, max(qcols, 8)


def _build_program(gathers, pair_chunks, waves, TC, qcols, repeat=1, ablate=()):
    nc = bacc.Bacc(
        "TRN2",
        target_bir_lowering=False,
        debug=False,
        enable_asserts=False,
        num_devices=N_CORES,
        num_swdge_queues=N_QUEUES,
    )
    x_d = nc.dram_tensor("x", [N_NODES, D_IN], _BF16, kind="ExternalInput").ap()
    xs_d = nc.dram_tensor("xs", [PAD_NODES, D_IN], _BF16, kind="ExternalInput").ap()
    idx_d = nc.dram_tensor("idx", [P, qcols], _I16, kind="ExternalInput").ap()
    dfl_d = nc.dram_tensor("dfl", [P, TC], _F32, kind="ExternalInput").ap()
    wt_d = nc.dram_tensor("wt", [P, D_OUT], _BF16, kind="ExternalInput").ap()
    bias_d = nc.dram_tensor("bias", [P, D_OUT], _BF16, kind="ExternalInput").ap()
    iota_d = nc.dram_tensor("iota", [P, DPAIR], _BF16, kind="ExternalInput").ap()
    ident_d = nc.dram_tensor("ident", [P, 2 * DPAIR], _BF16, kind="ExternalInput").ap()
    out_d = nc.dram_tensor("out", [PAD_NODES, D_OUT], _F32, kind="ExternalOutput").ap()

    with tile.TileContext(nc) as tc:
        with (
            tc.tile_pool(name="const", bufs=1) as cpool,
            tc.tile_pool(name="gather", bufs=2) as gpool,
            tc.tile_pool(name="xn", bufs=3) as xnpool,
            tc.tile_pool(name="oh", bufs=6) as ohpool,
            tc.tile_pool(name="ht", bufs=3) as htpool,
            tc.tile_pool(name="ot", bufs=3) as otpool,
            tc.tile_pool(name="psA", bufs=4, space="PSUM") as psA,
            tc.tile_pool(name="psB", bufs=3, space="PSUM") as psB,
        ):
            wt_t = cpool.tile([P, D_OUT], _BF16)
            nc.sync.dma_start(out=wt_t[:], in_=wt_d)
            bias_t = cpool.tile([P, D_OUT], _BF16)
            nc.sync.dma_start(out=bias_t[:], in_=bias_d)
            iota_t = cpool.tile([P, DPAIR], _BF16)
            nc.sync.dma_start(out=iota_t[:], in_=iota_d)
            ident_t = cpool.tile([P, 2 * DPAIR], _BF16)
            nc.sync.dma_start(out=ident_t[:], in_=ident_d)
            idx_t = cpool.tile([P, qcols], _I16)
            nc.sync.dma_start(out=idx_t[:], in_=idx_d)
            dfl_t = cpool.tile([P, TC], _F32)
            nc.sync.dma_start(out=dfl_t[:], in_=dfl_d)
            oh_const = None
            if "onehot" in ablate:
                oh_const = cpool.tile([P, DPAIR], _BF16)
                nc.vector.memset(oh_const[:], 0.0)

            for _rep in range(repeat):
              for w, wp in enumerate(waves):
                gts = {}
                for g in gathers:
                    if g["w"] != w:
                        continue
                    b = g["b"]
                    nch = g["nch"]
                    b0 = BOUNDS[b]
                    b1 = BOUNDS[b + 1] if b + 1 < N_BUCKETS else N_NODES
                    gt = gpool.tile([P, nch * P], _BF16, tag=f"g{b}")
                    if "gather" not in ablate:
                        nc.gpsimd.dma_gather(
                            gt[:].rearrange("p (c e) -> p c e", e=P),
                            x_d[b0:b1, :],
                            idx_t[:, g["qoff"] : g["qoff"] + nch * 8],
                            nch * P,
                            nch * P,
                            P,
                            single_packet=False,
                            queue_num=b % N_QUEUES,
                        )
                    gts[b] = gt
                for pi, p in enumerate(wp):
                    chunks = pair_chunks[p]
                    xn = xnpool.tile([P, DPAIR], _BF16)
                    for h in range(2):
                        r0 = p * DPAIR + h * P
                        nc.sync.dma_start(
                            out=xn[:, h * P : (h + 1) * P],
                            in_=xs_d[r0 : r0 + P, :],
                        )
                    psT = psA.tile([P, DPAIR], _F32)
                    # residual: psT[f, slot] = x_dest^T via identity matmuls
                    for h in range(2):
                        nc.tensor.matmul(
                            out=psT[:],
                            lhsT=xn[:, h * P : (h + 1) * P],
                            rhs=ident_t[:, h * DPAIR : (h + 1) * DPAIR],
                            start=(h == 0),
                            stop=(h == 1 and not chunks),
                        )
                    for k, (gi, lc, ci) in enumerate(chunks):
                        if "onehot" in ablate:
                            oh = oh_const
                        else:
                            oh = ohpool.tile([P, DPAIR], _BF16)
                            nc.vector.tensor_scalar(
                                out=oh[:],
                                in0=iota_t[:],
                                scalar1=dfl_t[:, ci : ci + 1],
                                scalar2=None,
                                op0=mybir.AluOpType.is_equal,
                            )
                        nc.tensor.matmul(
                            out=psT[:],
                            lhsT=gts[gathers[gi]["b"]][:, lc * P : (lc + 1) * P],
                            rhs=oh[:],
                            start=False,
                            stop=(k == len(chunks) - 1),
                        )
                    ht = htpool.tile([P, DPAIR], _BF16)
                    nc.scalar.copy(out=ht[:], in_=psT[:])
                    for h in range(2):
                        psO = psB.tile([P, D_OUT], _F32)
                        nc.tensor.matmul(
                            out=psO[:],
                            lhsT=ident_t[:, :P],
                            rhs=bias_t[:],
                            start=True,
                            stop=False,
                        )
                        nc.tensor.matmul(
                            out=psO[:],
                            lhsT=ht[:, h * P : (h + 1) * P],
                            rhs=wt_t[:],
                            start=False,
                            stop=True,
                        )
                        ot = otpool.tile([P, D_OUT], _F32)
                        nc.scalar.copy(out=ot[:], in_=psO[:])
                        r0 = p * DPAIR + h * P
                        if "outdma" not in ablate:
                            nc.sync.dma_start(out=out_d[r0 : r0 + P, :], in_=ot[:])
    nc.compile()
    return nc


def _prepare(inputs, repeat=1, ablate=()):
    """Host prep + program build: returns (nc, in_maps)."""
    import ml_dtypes

    x = np.ascontiguousarray(np.asarray(inputs["x"], dtype=np.float32))
    xb = np.ascontiguousarray(x.astype(ml_dtypes.bfloat16))
    W = np.asarray(inputs["W"], dtype=np.float32)
    b = np.asarray(inputs["b"], dtype=np.float32)

    idx_all, dfl_all, gathers, pair_chunks, waves, TC, qcols = _host_prep(
        inputs["edge_index"]
    )

    WT = np.ascontiguousarray(W.T.astype(ml_dtypes.bfloat16))
    bias_rep = np.ascontiguousarray(
        np.broadcast_to(b[None, :], (P, D_OUT)).astype(ml_dtypes.bfloat16)
    )
    iota = np.ascontiguousarray(
        np.broadcast_to(
            np.arange(DPAIR, dtype=np.float32)[None, :], (P, DPAIR)
        ).astype(ml_dtypes.bfloat16)
    )
    # ident[:, h*256:(h+1)*256][k, j] = 1 iff j == k + h*128
    ident = np.zeros((P, 2 * DPAIR), dtype=np.float32)
    ident[np.arange(P), np.arange(P)] = 1.0
    ident[np.arange(P), DPAIR + P + np.arange(P)] = 1.0

    in_maps = []
    for c in range(N_CORES):
        xs = np.zeros((PAD_NODES, D_IN), dtype=xb.dtype)
        xs[:NODES_PER_CORE] = xb[c * NODES_PER_CORE : (c + 1) * NODES_PER_CORE]
        in_maps.append(
            {
                "x": xb,
                "xs": np.ascontiguousarray(xs),
                "idx": np.ascontiguousarray(idx_all[c]),
                "dfl": np.ascontiguousarray(dfl_all[c]),
                "wt": WT,
                "bias": bias_rep,
                "iota": iota,
                "ident": np.ascontiguousarray(ident.astype(ml_dtypes.bfloat16)),
            }
        )

    nc = _build_program(
        gathers, pair_chunks, waves, TC, qcols, repeat=repeat, ablate=ablate
    )
    return nc, in_maps


def _run(inputs, trace=False, prepared=None):
    nc, in_maps = prepared if prepared is not None else _prepare(inputs)
    res = bass_utils.run_bass_kernel_spmd(
        nc, in_maps, core_ids=list(range(N_CORES)), trace=trace
    )
    out = np.concatenate(
        [res.results[c]["out"][:NODES_PER_CORE] for c in range(N_CORES)], axis=0
    )
    return out.astype(np.float32), res


def kernel(**inputs):
    out, _ = _run(inputs, trace=False)
    return out


# revision 34
# speedup vs baseline: 1.1080x; 1.0441x over previous
"""GPNConv (GNN message passing) Trainium2 Bass kernel.

  agg = segment_sum(x[col], row, N)        # [N, 128]
  out = (x + agg) @ W.T + b                # [N, 512]

Sharding: destination nodes split across 8 cores (12500 each); no
cross-core communication. Per core, edges are grouped by 256-node
destination window ("pair") and by 25000-row source bucket (dma_gather
indices are int16). Each core bulk-gathers its neighbor rows x[col]
from a replicated x via dma_gather (4 SWDGE queues, one per bucket),
segment-sums them with one-hot matmuls on the PE (one-hot built on DVE
from dest slots via is_equal against an iota row, all bf16), adds the
residual via PE identity matmuls from a per-core natural x-shard, folds
the bias in as a rank-1 PE matmul, copies PSUM->SBUF on the scalar
engine, applies the 128->512 linear from bf16 tiles, and DMAs each
output block straight from PSUM.

The chunk schedule (how many 128-edge chunks each (pair, bucket) group
gets) is data-dependent but made uniform across cores by taking the max
over cores, so one SPMD program serves all 8 cores.
"""

import numpy as np

import concourse.bass as bass
import concourse.mybir as mybir
import concourse.tile as tile
from concourse import bacc
from concourse import bass_utils

P = 128
N_NODES = 100000
D_IN = 128
D_OUT = 512
N_CORES = 8
NODES_PER_CORE = N_NODES // N_CORES             # 12500
DPAIR = 256                                      # dest window (psum free dim)
PAIRS_PER_CORE = (NODES_PER_CORE + DPAIR - 1) // DPAIR  # 49
PAD_NODES = PAIRS_PER_CORE * DPAIR               # 12544
WAVE_PAIRS = 7                                   # pairs per gather wave
N_WAVES = (PAIRS_PER_CORE + WAVE_PAIRS - 1) // WAVE_PAIRS  # 7
N_BUCKETS = 4
BOUNDS = (0, 28800, 57600, 86400)                # source bucket starts (int16 range)
N_QUEUES = 4                                     # SWDGE queues (one per bucket)
PAD_SLOT = 300.0                                 # one-hot slot matching nothing

_F32 = mybir.dt.float32
_BF16 = mybir.dt.bfloat16
_I16 = mybir.dt.int16


def _host_prep(edge_index):
    """Group edges by (core, pair, bucket); build uniform chunk schedule,
    int16 gather-index array and bf16 dest-slot array per core."""
    row = np.asarray(edge_index[0], dtype=np.int64)
    col = np.asarray(edge_index[1], dtype=np.int64)

    core = row // NODES_PER_CORE
    local = row % NODES_PER_CORE
    pair = local // DPAIR                         # 0..48
    pslot = local % DPAIR                         # 0..255
    bounds = np.asarray(BOUNDS)
    bucket = np.searchsorted(bounds, col, side="right") - 1  # 0..3
    brel = (col - bounds[bucket]).astype(np.int16)

    key = (core * PAIRS_PER_CORE + pair) * N_BUCKETS + bucket
    ngroups = N_CORES * PAIRS_PER_CORE * N_BUCKETS
    counts = np.bincount(key, minlength=ngroups).reshape(
        N_CORES, PAIRS_PER_CORE, N_BUCKETS
    )
    budget = -(-counts.max(axis=0) // P)          # [PAIRS, NB] ceil

    order = np.argsort(key, kind="stable")
    brel_s = brel[order]
    pslot_s = pslot[order]
    key_s = key[order]
    starts = np.searchsorted(key_s, np.arange(ngroups + 1))

    waves = [
        list(range(w * WAVE_PAIRS, min((w + 1) * WAVE_PAIRS, PAIRS_PER_CORE)))
        for w in range(N_WAVES)
    ]
    gathers = []      # per (w,b): dict(nch, qoff, coff, b, w)
    pair_chunks = {p: [] for p in range(PAIRS_PER_CORE)}
    ci = 0            # global chunk counter
    qcols = 0         # idx tile columns consumed (num_idxs/16 each)
    for w, wp in enumerate(waves):
        for b in range(N_BUCKETS):
            nch = int(sum(budget[p][b] for p in wp))
            if nch == 0:
                continue
            gathers.append(dict(w=w, b=b, nch=nch, qoff=qcols, coff=ci))
            lc = 0
            for p in wp:
                for j in range(int(budget[p][b])):
                    pair_chunks[p].append((len(gathers) - 1, lc, ci))
                    lc += 1
                    ci += 1
            qcols += nch * 8  # (nch*128 idxs)/16
    TC = max(ci, 1)

    idx_all = np.zeros((N_CORES, 16, max(qcols, 8)), dtype=np.int16)
    dfl_all = np.full((N_CORES, P, TC), PAD_SLOT, dtype=np.float32)
    for c in range(N_CORES):
        for g in gathers:
            w, b = g["w"], g["b"]
            lc = 0
            for p in waves[w]:
                gk = (c * PAIRS_PER_CORE + p) * N_BUCKETS + b
                b0, b1 = starts[gk], starts[gk + 1]
                n = b1 - b0
                kb = int(budget[p][b])
                assert n <= kb * P
                if n:
                    i = (lc + np.arange(n) // P) * P + np.arange(n) % P
                    idx_all[c, i % 16, g["qoff"] + i // 16] = brel_s[b0:b1]
                    dfl_all[c, np.arange(n) % P, g["coff"] + lc + np.arange(n) // P] = (
                        pslot_s[b0:b1]
                    )
                lc += kb
    # replicate idx rows to 128 partitions (8 Q7 cores x 16-partition stripes)
    idx_all = np.tile(idx_all, (1, 8, 1))
    return idx_all, dfl_all, gathers, pair_chunks, waves, TC, max(qcols, 8)


def _build_program(gathers, pair_chunks, waves, TC, qcols, repeat=1, ablate=()):
    nc = bacc.Bacc(
        "TRN2",
        target_bir_lowering=False,
        debug=False,
        enable_asserts=False,
        num_devices=N_CORES,
        num_swdge_queues=N_QUEUES,
    )
    x_d = nc.dram_tensor("x", [N_NODES, D_IN], _BF16, kind="ExternalInput").ap()
    xs_d = nc.dram_tensor("xs", [PAD_NODES, D_IN], _BF16, kind="ExternalInput").ap()
    idx_d = nc.dram_tensor("idx", [P, qcols], _I16, kind="ExternalInput").ap()
    dfl_d = nc.dram_tensor("dfl", [P, TC], _F32, kind="ExternalInput").ap()
    wt_d = nc.dram_tensor("wt", [P, D_OUT], _BF16, kind="ExternalInput").ap()
    bias_d = nc.dram_tensor("bias", [P, D_OUT], _BF16, kind="ExternalInput").ap()
    iota_d = nc.dram_tensor("iota", [P, DPAIR], _BF16, kind="ExternalInput").ap()
    ident_d = nc.dram_tensor("ident", [P, 2 * DPAIR], _BF16, kind="ExternalInput").ap()
    out_d = nc.dram_tensor("out", [PAD_NODES, D_OUT], _F32, kind="ExternalOutput").ap()

    with tile.TileContext(nc) as tc:
        with (
            tc.tile_pool(name="const", bufs=1) as cpool,
            tc.tile_pool(name="gather", bufs=2) as gpool,
            tc.tile_pool(name="xn", bufs=3) as xnpool,
            tc.tile_pool(name="oh", bufs=6) as ohpool,
            tc.tile_pool(name="ht", bufs=3) as htpool,
            tc.tile_pool(name="ot", bufs=3) as otpool,
            tc.tile_pool(name="psA", bufs=4, space="PSUM") as psA,
            tc.tile_pool(name="psB", bufs=3, space="PSUM") as psB,
        ):
            wt_t = cpool.tile([P, D_OUT], _BF16)
            nc.sync.dma_start(out=wt_t[:], in_=wt_d)
            bias_t = cpool.tile([P, D_OUT], _BF16)
            nc.sync.dma_start(out=bias_t[:], in_=bias_d)
            iota_t = cpool.tile([P, DPAIR], _BF16)
            nc.sync.dma_start(out=iota_t[:], in_=iota_d)
            ident_t = cpool.tile([P, 2 * DPAIR], _BF16)
            nc.sync.dma_start(out=ident_t[:], in_=ident_d)
            idx_t = cpool.tile([P, qcols], _I16)
            nc.sync.dma_start(out=idx_t[:], in_=idx_d)
            dfl_t = cpool.tile([P, TC], _F32)
            nc.sync.dma_start(out=dfl_t[:], in_=dfl_d)
            oh_const = None
            if "onehot" in ablate:
                oh_const = cpool.tile([P, DPAIR], _BF16)
                nc.vector.memset(oh_const[:], 0.0)

            for _rep in range(repeat):
              for w, wp in enumerate(waves):
                gts = {}
                for g in gathers:
                    if g["w"] != w:
                        continue
                    b = g["b"]
                    nch = g["nch"]
                    b0 = BOUNDS[b]
                    b1 = BOUNDS[b + 1] if b + 1 < N_BUCKETS else N_NODES
                    gt = gpool.tile([P, nch * P], _BF16, tag=f"g{b}")
                    if "gather" not in ablate:
                        nc.gpsimd.dma_gather(
                            gt[:].rearrange("p (c e) -> p c e", e=P),
                            x_d[b0:b1, :],
                            idx_t[:, g["qoff"] : g["qoff"] + nch * 8],
                            nch * P,
                            nch * P,
                            P,
                            single_packet=False,
                            queue_num=b % N_QUEUES,
                        )
                    else:
                        nc.sync.dma_start(
                            out=gt[:],
                            in_=x_d[0 : nch * P, :].rearrange(
                                "(p a) e -> p (a e)", p=P
                            ),
                        )
                    gts[b] = gt
                # one natural-layout x-shard load per wave: block a holds
                # rows wp[0]*DPAIR + a*128 .. +128 in its 128 partitions
                nblk = 2 * len(wp)
                xnw = xnpool.tile([P, nblk * P], _BF16)
                nc.sync.dma_start(
                    out=xnw[:].rearrange("p (a e) -> p a e", e=P),
                    in_=xs_d[wp[0] * DPAIR : wp[0] * DPAIR + nblk * P, :].rearrange(
                        "(a p) e -> p a e", p=P
                    ),
                )
                for pi, p in enumerate(wp):
                    chunks = pair_chunks[p]
                    xn = xnw[:, pi * DPAIR : (pi + 1) * DPAIR]
                    psT = psA.tile([P, DPAIR], _F32)
                    mm_chunks = [] if "matmul" in ablate else chunks
                    # residual: psT[f, slot] = x_dest^T via identity matmuls
                    for h in range(2):
                        nc.tensor.matmul(
                            out=psT[:],
                            lhsT=xn[:, h * P : (h + 1) * P],
                            rhs=ident_t[:, h * DPAIR : (h + 1) * DPAIR],
                            start=(h == 0),
                            stop=(h == 1 and not mm_chunks),
                        )
                    for k, (gi, lc, ci) in enumerate(chunks):
                        if "onehot" in ablate:
                            oh = oh_const
                        else:
                            oh = ohpool.tile([P, DPAIR], _BF16)
                            nc.vector.tensor_scalar(
                                out=oh[:],
                                in0=iota_t[:],
                                scalar1=dfl_t[:, ci : ci + 1],
                                scalar2=None,
                                op0=mybir.AluOpType.is_equal,
                            )
                        if "matmul" not in ablate:
                            nc.tensor.matmul(
                                out=psT[:],
                                lhsT=gts[gathers[gi]["b"]][:, lc * P : (lc + 1) * P],
                                rhs=oh[:],
                                start=False,
                                stop=(k == len(chunks) - 1),
                            )
                    ht = htpool.tile([P, DPAIR], _BF16)
                    nc.scalar.copy(out=ht[:], in_=psT[:])
                    ot = otpool.tile([P, 2 * D_OUT], _F32)
                    for h in range(2):
                        psO = psB.tile([P, D_OUT], _F32)
                        nc.tensor.matmul(
                            out=psO[:],
                            lhsT=ident_t[:, :P],
                            rhs=bias_t[:],
                            start=True,
                            stop=False,
                        )
                        nc.tensor.matmul(
                            out=psO[:],
                            lhsT=ht[:, h * P : (h + 1) * P],
                            rhs=wt_t[:],
                            start=False,
                            stop=True,
                        )
                        nc.scalar.copy(
                            out=ot[:, h * D_OUT : (h + 1) * D_OUT], in_=psO[:]
                        )
                    if "outdma" not in ablate:
                        nc.sync.dma_start(
                            out=out_d[p * DPAIR : (p + 1) * DPAIR, :].rearrange(
                                "(h k) e -> k h e", h=2
                            ),
                            in_=ot[:].rearrange("k (h e) -> k h e", e=D_OUT),
                        )
    nc.compile()
    return nc


def _prepare(inputs, repeat=1, ablate=()):
    """Host prep + program build: returns (nc, in_maps)."""
    import ml_dtypes

    x = np.ascontiguousarray(np.asarray(inputs["x"], dtype=np.float32))
    xb = np.ascontiguousarray(x.astype(ml_dtypes.bfloat16))
    W = np.asarray(inputs["W"], dtype=np.float32)
    b = np.asarray(inputs["b"], dtype=np.float32)

    idx_all, dfl_all, gathers, pair_chunks, waves, TC, qcols = _host_prep(
        inputs["edge_index"]
    )

    WT = np.ascontiguousarray(W.T.astype(ml_dtypes.bfloat16))
    bias_rep = np.ascontiguousarray(
        np.broadcast_to(b[None, :], (P, D_OUT)).astype(ml_dtypes.bfloat16)
    )
    iota = np.ascontiguousarray(
        np.broadcast_to(
            np.arange(DPAIR, dtype=np.float32)[None, :], (P, DPAIR)
        ).astype(ml_dtypes.bfloat16)
    )
    # ident[:, h*256:(h+1)*256][k, j] = 1 iff j == k + h*128
    ident = np.zeros((P, 2 * DPAIR), dtype=np.float32)
    ident[np.arange(P), np.arange(P)] = 1.0
    ident[np.arange(P), DPAIR + P + np.arange(P)] = 1.0

    in_maps = []
    for c in range(N_CORES):
        xs = np.zeros((PAD_NODES, D_IN), dtype=xb.dtype)
        xs[:NODES_PER_CORE] = xb[c * NODES_PER_CORE : (c + 1) * NODES_PER_CORE]
        in_maps.append(
            {
                "x": xb,
                "xs": np.ascontiguousarray(xs),
                "idx": np.ascontiguousarray(idx_all[c]),
                "dfl": np.ascontiguousarray(dfl_all[c]),
                "wt": WT,
                "bias": bias_rep,
                "iota": iota,
                "ident": np.ascontiguousarray(ident.astype(ml_dtypes.bfloat16)),
            }
        )

    nc = _build_program(
        gathers, pair_chunks, waves, TC, qcols, repeat=repeat, ablate=ablate
    )
    return nc, in_maps


def _run(inputs, trace=False, prepared=None):
    nc, in_maps = prepared if prepared is not None else _prepare(inputs)
    res = bass_utils.run_bass_kernel_spmd(
        nc, in_maps, core_ids=list(range(N_CORES)), trace=trace
    )
    out = np.concatenate(
        [res.results[c]["out"][:NODES_PER_CORE] for c in range(N_CORES)], axis=0
    )
    return out.astype(np.float32), res


def kernel(**inputs):
    out, _ = _run(inputs, trace=False)
    return out
